# revision 61
# baseline (speedup 1.0000x reference)
"""Trainium2 distributed kernel for nn_Attention (dense transformer attention block).

Strategy (8 NeuronCores, tensor-parallel over heads, batch-pipelined,
fine-grained weave):
  - Host pre-transposes x_norm -> X^T [C, B*T] (bf16) and slices Wqkv columns
    per core (2 heads/core, deinterleaved RoPE feature order). RoPE sin/cos
    tables precomputed host-side (sin is sign-folded for full-128 ops).
  - Emission is piece-granular: stage-1 (next batch) and out-projection
    (previous batch) units are generators whose ~0.5us pieces are woven
    between every attention S-matmul and its dependent PV-matmul, so the
    exp (ACT) latency never stalls the in-order tensor queue.
  - stage1(bb): per 256-row chunk: Q^T/K^T (head-major, D on partitions,
    packed q|k PSUM) + V natural (packed PSUM). Bias epilogues on gpsimd,
    V copy on vector, RoPE on vector (scalar engine is reserved for exp).
  - attention: S^T flash form without max-subtraction: S^T tile -> exp (ACT,
    scaled) -> P^T bf16 -> V^T@P^T accumulate (PSUM) + rowsums (vector, bf16)
    -> gpsimd partition allreduce -> reciprocal -> normalize into a per-chunk
    staging tile streamed straight to the a2a dram tile.
  - Per-(batch, head) AllToAll (0.5 MiB bf16) issued as soon as that head
    finishes; the per-head x2 gather DMA is issued immediately after the
    collective so out-projection inputs land long before they are needed.
  - Wout is resident in SBUF (loaded once, 4 tiles spread over 4 rings).
  - Tail: out-projection of the last batch is split even/odd-kt: even kts
    depend only on head 0's collective and cover the final collective's
    flight; odd kts complete the contraction once it lands.
  - Startup: first x chunk + wq land on separate rings before anything else;
    cos/sin/bo and Wout loads are deferred behind them.
  - Host reassembles the per-(core, batch) row pieces -> [B, T, C] fp32.
"""

import numpy as np
import ml_dtypes

import concourse.bass as bass
import concourse.bass_isa as bass_isa
import concourse.mybir as mybir
import concourse.tile as tile
from concourse import bacc
from concourse.bass_utils import run_bass_kernel_spmd


N_CORES = 8
B, T, C = 4, 2048, 2048
H, D = 16, 128
ROPE_BASE = 10000.0

BF16 = mybir.dt.bfloat16
F32 = mybir.dt.float32
NPBF16 = ml_dtypes.bfloat16


class Ctx:
    """Bag of tiles/pools/params shared by the emission helpers."""
    pass


def _alloc_qkv(cx, bb):
    """Rolling per-batch QKV tiles (pool bufs=2 -> 2 batches in flight)."""
    p = cx.p
    cx.qkv[bb] = (
        cx.qkvpool.tile([128, p["HL"], p["t"]], BF16, tag="qT", name=f"qT{bb}"),
        cx.qkvpool.tile([128, p["HL"], p["t"]], BF16, tag="kT", name=f"kT{bb}"),
        cx.qkvpool.tile([128, p["t"] // 128, p["HD"]], BF16, tag="v", name=f"v{bb}"),
    )


def _s1_gen(nc, cx, bb, rc):
    """Stage-1 unit (512 rows) as a generator of ~0.5us pieces. Target-outer:
    each of Q0/Q1/K0/K1 accumulates its own 1-bank PSUM tile over all kt
    (512-col matmuls, half the instruction count of the 256-row form), then
    V in two packed psum tiles. Q targets run first so batch 0 can start on
    wq + x0 alone."""
    p = cx.p
    RC, KT, HL = p["RC"], p["KT"], p["HL"]
    t = p["t"]
    r0g = bb * t + rc * RC
    t0 = rc * RC
    qT_t, kT_t, v_t = cx.qkv[bb]
    rcg = r0g // RC
    xt = cx.xin.tile([128, KT, RC], BF16, tag="xt")
    # each chunk is fetched as two parallel halves (one per ring) to halve
    # its arrival latency; batch 0's unit 2 rides the gpsimd ring so three
    # rings feed the unwoven prologue
    xsrc = cx.xB[rcg * 128:(rcg + 1) * 128, :].rearrange(
        "p (kt r) -> p kt r", kt=KT)
    hk = KT // 2
    rings = ((nc.gpsimd, nc.gpsimd) if (bb == 0 and rc == 2)
             else (cx.dma3[0], cx.dma3[1]))
    if bb == 0 and rc == 0:
        # quarter-granular so the first Q matmul starts on kt 0-3 alone
        qk_ = KT // 4
        for qi in range(4):
            rings[qi % 2].dma_start(out=xt[:, qi * qk_:(qi + 1) * qk_, :],
                                    in_=xsrc[:, qi * qk_:(qi + 1) * qk_, :])
    else:
        rings[0].dma_start(out=xt[:, 0:hk, :], in_=xsrc[:, 0:hk, :])
        rings[1].dma_start(out=xt[:, hk:KT, :], in_=xsrc[:, hk:KT, :])
    yield
    for which, w_sb, dstt, bias_sb in (
            (0, cx.wq_sb, qT_t, cx.bq_sb), (1, cx.wk_sb, kT_t, cx.bk_sb)):
        for hm in range(HL):
            ps = cx.s1ps.tile([128, RC], F32, tag="s1ps",
                              name=f"ps{bb}_{rc}_{which}_{hm}")
            for kt in range(KT):
                nc.tensor.matmul(ps, lhsT=w_sb[:, kt, hm * 128:(hm + 1) * 128],
                                 rhs=xt[:, kt, :],
                                 start=(kt == 0), stop=(kt == KT - 1))
                if kt % 2 == 1:
                    yield
            nc.scalar.activation(out=dstt[:, hm, t0:t0 + RC], in_=ps,
                                 func=mybir.ActivationFunctionType.Identity,
                                 bias=bias_sb[:, hm:hm + 1], scale=1.0)
    # ---- V natural: two psum tiles, each packing two 128-row blocks ----
    for vg in range(RC // 256):
        psv = cx.s1ps.tile([128, 2 * p["HD"]], F32, tag="s1ps",
                           name=f"psv{bb}_{rc}_{vg}")
        for kt in range(KT):
            for rr in range(2):
                rs = 2 * vg + rr
                nc.tensor.matmul(psv[:, rr * p["HD"]:(rr + 1) * p["HD"]],
                                 lhsT=xt[:, kt, rs * 128:(rs + 1) * 128],
                                 rhs=cx.wv_sb[:, kt, :],
                                 start=(kt == 0 and rr == 0), stop=(kt == KT - 1))
            if kt % 2 == 1:
                yield
        for rr in range(2):
            rt_ = (t0 // 128) + 2 * vg + rr
            nc.scalar.activation(out=v_t[:, rt_, :],
                                 in_=psv[:, rr * p["HD"]:(rr + 1) * p["HD"]],
                                 func=mybir.ActivationFunctionType.Copy, scale=1.0)
        yield
    # ---- RoPE in place on this 512-row span (both heads, q and k) ----
    # x0' = x0*cos - x1*sin ; x1' = x1*cos + x0*sin
    # (sin sign-folded host-side: rows 0-63 = +sin, rows 64-127 = -sin)
    for hm in range(HL):
        for res in (qT_t, kT_t):
            x = res[:, hm, t0:t0 + RC]
            rt = cx.ropetmp.tile([128, RC], BF16, tag="rt")
            nc.vector.tensor_mul(rt[0:64, :], x[64:128, :], cx.sin_sb[64:128, t0:t0 + RC])
            yield
            nc.vector.tensor_mul(rt[64:128, :], x[0:64, :], cx.sin_sb[0:64, t0:t0 + RC])
            nc.vector.tensor_mul(x, x, cx.cos_sb[:, t0:t0 + RC])
            nc.vector.tensor_add(x, x, rt)
        yield


S1_PIECES = 57   # yields per _s1_gen: 1 xt + 32 qk + 18 v + 6 rope
OP_PIECES = 3    # yields per full _outproj_gen (16 kts, yield every 4th, not last)


def _attn_gen(nc, cx, bb, hm, c):
    """Attention unit generator: yields after S(+exp) and after PV per jt."""
    p = cx.p
    t, d = p["t"], p["d"]
    SCALE = p["SCALE"]
    tq0 = c * 512
    jt_max = 4 * (c + 1)
    qT_t, kT_t, v_t = cx.qkv[bb]
    qT_h = qT_t[:, hm, :]
    kT_h = kT_t[:, hm, :]
    psum_o = cx.apsum.tile([128, 512], F32, tag="po", name=f"po{bb}_{hm}_{c}")
    # bf16 rowsum accumulation: DVE runs 2x mode; allreduce upcasts to f32
    rs_d = cx.rsp.tile([128, 512], BF16, tag="rsd", name=f"rsd{bb}_{hm}_{c}")
    for jt in range(jt_max):
        off = max(0, jt * 128 - tq0)
        st = cx.spsum.tile([128, 512], F32, tag="st", name=f"st{bb}_{hm}_{c}_{jt}")
        nc.tensor.matmul(st[:, off:512],
                         lhsT=kT_h[:, jt * 128:(jt + 1) * 128],
                         rhs=qT_h[:, tq0 + off:tq0 + 512],
                         start=True, stop=True)
        pT = cx.att.tile([128, 512], BF16, tag="pT", name=f"pT{bb}_{hm}_{c}_{jt}")
        nc.scalar.activation(out=pT[:, off:512], in_=st[:, off:512],
                             func=mybir.ActivationFunctionType.Exp, scale=SCALE)
        if jt * 128 >= tq0:
            nc.gpsimd.tensor_mul(pT[:, off:off + 128], pT[:, off:off + 128],
                                 cx.maskU_sb)
        yield  # fillers injected here hide the exp latency
        nc.tensor.matmul(psum_o[:, off:512],
                         lhsT=v_t[:, jt, hm * d:(hm + 1) * d],
                         rhs=pT[:, off:512],
                         start=(jt == 0), stop=(jt == jt_max - 1))
        if jt == 0:
            nc.vector.tensor_copy(rs_d, pT)
        else:
            nc.vector.tensor_add(rs_d[:, off:512], rs_d[:, off:512], pT[:, off:512])
        yield
    rs_red = cx.rsrp.tile([128, 512], F32, tag="rsr", name=f"rsr{bb}_{hm}_{c}")
    nc.gpsimd.partition_all_reduce(rs_red, rs_d, 128, bass_isa.ReduceOp.add)
    nc.vector.reciprocal_approx_fast(out=rs_red, in_=rs_red)
    oc = cx.ocp.tile([128, 512], BF16, tag="oc", name=f"oc{bb}_{hm}_{c}")
    nc.vector.tensor_mul(oc, psum_o, rs_red)
    nc.vector.tensor_scalar_add(oc, oc, cx.bvh_sb[:, hm:hm + 1])
    # stream this chunk's two slot-columns into the a2a dram tile now, so the
    # payload write is off the collective's critical path. These moves are
    # 512B-packet-bound (~10us each), so round-robin them over three rings.
    eng = (nc.gpsimd, nc.sync, nc.scalar)[(hm * 4 + c) % 3]
    eng.dma_start(
        out=cx.a2a_ins[(bb, hm)][2 * c * 128:(2 * c + 2) * 128, :].rearrange(
            "(sl dd) r -> dd sl r", dd=p["d"]),
        in_=oc.rearrange("p (sl r) -> p sl r", sl=2))


def _issue_a2a(nc, cx, bb, hm):
    """Issue the AllToAll for (bb, hm). The dependent x2 gather is queued and
    flushed at the NEXT collective boundary: a gather issued immediately
    would head-of-line-block its DMA rings for the whole collective flight,
    delaying the next head's payload streams."""
    p = cx.p
    n_cores = N_CORES
    # any pending gather's collective has had a full head's compute to land
    _flush_gathers(nc, cx)
    nc.gpsimd.collective_compute(
        "AllToAll", mybir.AluOpType.bypass,
        replica_groups=[list(range(n_cores))],
        ins=[cx.a2a_ins[(bb, hm)][:, :].opt()],
        outs=[cx.a2a_outs[(bb, hm)][:, :].opt()],
    )
    cx.pending_gathers.append((bb, hm))
    if hm == 1:
        _flush_gathers(nc, cx)


def _flush_gathers(nc, cx):
    """Emit the x2 gather DMAs for all pending collectives (their flights
    have ended, or nothing time-critical sits behind them on the ring).
    Each gather is split across two rings: it moves 512B packets, so one
    ring alone takes ~15us."""
    p = cx.p
    seg, KT, HL = p["t"] // N_CORES, p["KT"], p["HL"]
    for bb, hm in cx.pending_gathers:
        x2h = cx.x2pool.tile([128, KT // HL, seg], BF16, tag="x2",
                             name=f"x2_{bb}_{hm}")
        src = cx.a2a_outs[(bb, hm)][:, :].rearrange("(sl p) r -> p sl r", p=128)
        nc.gpsimd.dma_start(out=x2h[:, 0:3, :], in_=src[:, 0:3, :])
        nc.sync.dma_start(out=x2h[:, 3:6, :], in_=src[:, 3:6, :])
        nc.scalar.dma_start(out=x2h[:, 6:8, :], in_=src[:, 6:8, :])
        cx.x2[(bb, hm)] = x2h
    cx.pending_gathers = []


def _outproj_gen(nc, cx, bb, nn, m, half=None, pool=None, stage_pool=None):
    """Out-projection unit generator: one (batch, 512-outcol, 128-row) chunk.

    half=None: full 16-kt contraction. half=0: even kts (head-0 channels,
    depends only on that head's collective), parked in SBUF. half=1: odd
    kts, added to the parked half, then written out."""
    p = cx.p
    KT, HL = p["KT"], p["HL"]
    seg = p["t"] // N_CORES
    kts = (list(range(0, KT, 2)) + list(range(1, KT, 2))
           if half is None else list(range(half, KT, 2)))
    # tag matches the host pool's resident tag so no extra bank is reserved
    ps3 = (pool or cx.oppj).tile([128, 512], F32,
                                 tag="po" if pool is not None else "pj",
                                 name=f"ps3{bb}_{nn}_{m}_{half}")
    for i, kt in enumerate(kts):
        x2h = cx.x2[(bb, kt % HL)]
        nc.tensor.matmul(ps3, lhsT=x2h[:, kt // HL, m * 128:(m + 1) * 128],
                         rhs=cx.wo_sb[nn][:, kt, :],
                         start=(i == 0), stop=(i == len(kts) - 1))
        if i % 4 == 3 and i != len(kts) - 1:
            yield
    # bout is added host-side, so the PSUM drain is a plain copy that can run
    # on scalar (keeping the outproj-PSUM recycle off the vector queue)
    if half == 0:
        o3 = cx.o3pool.tile([128, 512], BF16, tag="o3", name=f"o3{bb}_{nn}_{m}")
        cx.op_tiles[(bb, nn, m, "o3")] = o3
        nc.scalar.activation(out=o3, in_=ps3,
                             func=mybir.ActivationFunctionType.Copy, scale=1.0)
        return
    if half == 1:
        o3 = cx.op_tiles[(bb, nn, m, "o3")]
        nc.vector.tensor_add(o3, o3, ps3)
    else:
        o3 = (stage_pool or cx.o3pool).tile(
            [128, 512], BF16, tag="oc" if stage_pool is not None else "o3",
            name=f"o3{bb}_{nn}_{m}")
        nc.scalar.activation(out=o3, in_=ps3,
                             func=mybir.ActivationFunctionType.Copy, scale=1.0)
    cx.outdma[(bb * 2 + m) % 2].dma_start(
        out=cx.out[bb * seg + m * 128:bb * seg + (m + 1) * 128,
                   nn * 512:(nn + 1) * 512], in_=o3)


def _drain(gen):
    for _ in gen:
        pass


def _window(nc, cx, bb, b):
    """Emit one pipeline window: attention(bb) woven with stage1(bb+1) and
    outproj fillers; collectives at head boundaries."""
    p = cx.p
    HL = p["HL"]
    # flush the previous window's x2 gathers: those collectives have landed,
    # so the gathers won't block the ring
    _flush_gathers(nc, cx)
    # attention units: big chunk (c=3) first so each head's last-finished
    # chunk is the small one, shortening the chain into its collective
    attn_units = [(hm, cc) for hm in range(HL) for cc in (3, 2, 1, 0)]
    n_slots = sum(2 * 4 * (cc + 1) + 1 for _, cc in attn_units)

    s1_gens = ([_s1_gen(nc, cx, bb + 1, rc) for rc in range(p["t"] // p["RC"])]
               if bb + 1 < b else [])
    # outproj filler assignment: window 1 takes batch 0; the second-to-last
    # window is left without outproj so the LAST window (which has no stage-1
    # filler) gets two batches' worth of filler matmuls to hide exp latency
    if bb == 0 or bb == b - 2:
        op_gens = []
    elif bb < b - 2:
        op_gens = [_outproj_gen(nc, cx, bb - 1, nn, m)
                   for nn in range(4) for m in range(2)]
    else:
        op_gens = [_outproj_gen(nc, cx, bbx, nn, m)
                   for bbx in (bb - 2, bb - 1)
                   for nn in range(4) for m in range(2)]
    prim = list(s1_gens)
    sec = list(op_gens)

    def pull(q):
        while q:
            try:
                next(q[0])
                return True
            except StopIteration:
                q.pop(0)
        return False

    # Front-load stage-1: exhaust it by ~70% of the window so the last
    # units' epilogues/RoPE have slack before the next window's attention
    # consumes qT/kT/v. Outproj is spread evenly (after a short warmup so
    # its first matmuls never head the tensor queue before inputs land).
    prim_rate = S1_PIECES * len(s1_gens) / (0.6 * n_slots)
    sec_rate = OP_PIECES * len(op_gens) / max(1, n_slots - 24)
    state = {"prim_debt": 0.0, "sec_debt": -24.0 * sec_rate}

    def pace():
        state["prim_debt"] += prim_rate
        state["sec_debt"] += sec_rate
        while state["prim_debt"] >= 1.0 and pull(prim):
            state["prim_debt"] -= 1.0
        if not prim:
            state["prim_debt"] = 0.0
        while state["sec_debt"] >= 1.0 and pull(sec):
            state["sec_debt"] -= 1.0
        if not sec:
            state["sec_debt"] = 0.0

    def drive(units, interleave=False):
        gens = [_attn_gen(nc, cx, bb, hm, cc) for hm, cc in units]
        if interleave:
            live = list(gens)
            while live:
                nxt = []
                for g in live:
                    try:
                        next(g)
                        pace()
                        nxt.append(g)
                    except StopIteration:
                        pass
                live = nxt
        else:
            for g in gens:
                while True:
                    try:
                        next(g)
                    except StopIteration:
                        break
                    pace()

    for ui, (hm, cc) in enumerate(attn_units):
        drive([(hm, cc)])
        if cc == 0:
            _issue_a2a(nc, cx, bb, hm)
        elif ui == 4:
            # head 0's collective has had a full unit's compute to land
            _flush_gathers(nc, cx)
    # drain leftover fillers before the next window
    for gg in prim + sec:
        _drain(gg)


def build_nc(b=B, t=T, c=C, h=H, d=D, n_cores=N_CORES):
    HL = h // n_cores          # heads per core
    R = b * t                  # total rows
    RS = R // n_cores          # rows per core overall (output)
    RC = 512                   # row-chunk for stage 1
    p = dict(b=b, t=t, c=c, h=h, d=d, HL=HL, R=R, RS=RS, RC=RC,
             KT=c // 128, HD=HL * d, SCALE=1.0 / float(np.sqrt(d)))
    seg = t // n_cores
    n_rc = t // RC             # stage-1 units per batch

    nc = bacc.Bacc(None, target_bir_lowering=False, debug=False,
                   num_devices=n_cores)

    cx = Ctx()
    cx.p = p
    cx.xB = nc.declare_dram_parameter("xB", [(R // RC) * 128, (c // 128) * RC], BF16, isOutput=False)
    # weights host-pretiled to [128, KT*HD] so each load is 128 contiguous
    # 8KB descriptors instead of 2048 x 512B packets
    wq = nc.declare_dram_parameter("wq", [128, p["KT"] * p["HD"]], BF16, isOutput=False)
    wk = nc.declare_dram_parameter("wk", [128, p["KT"] * p["HD"]], BF16, isOutput=False)
    wv = nc.declare_dram_parameter("wv", [128, p["KT"] * p["HD"]], BF16, isOutput=False)
    bq = nc.declare_dram_parameter("bq", [128, HL], F32, isOutput=False)
    bk = nc.declare_dram_parameter("bk", [128, HL], F32, isOutput=False)
    bvh = nc.declare_dram_parameter("bvh", [128, HL], F32, isOutput=False)
    cx.wo = nc.declare_dram_parameter("wo", [(c // 512) * 128, (c // 128) * 512], BF16, isOutput=False)
    cosT = nc.declare_dram_parameter("cosT", [128, t], BF16, isOutput=False)
    sinT = nc.declare_dram_parameter("sinT", [128, t], BF16, isOutput=False)
    maskc = nc.declare_dram_parameter("maskc", [128, 128], BF16, isOutput=False)
    cx.out = nc.declare_dram_parameter("out", [RS, c], BF16, isOutput=True)

    with tile.TileContext(nc) as tc:
        with (
            tc.tile_pool(name="consts", bufs=1) as consts,
            tc.tile_pool(name="qkvres", bufs=2) as qkvres,
            tc.tile_pool(name="dram", bufs=1, space="DRAM") as dram,
            tc.tile_pool(name="xin", bufs=2) as xin,
            tc.tile_pool(name="ropet", bufs=1) as ropetmp,
            tc.tile_pool(name="s1ps", bufs=3, space="PSUM") as s1ps,
            tc.tile_pool(name="spsum", bufs=2, space="PSUM") as spsum,
            tc.tile_pool(name="apsum", bufs=2, space="PSUM") as apsum,
            tc.tile_pool(name="oppj", bufs=1, space="PSUM") as oppj,
            tc.tile_pool(name="att", bufs=3) as att,
            tc.tile_pool(name="rsp", bufs=2) as rsp,
            tc.tile_pool(name="rsr", bufs=1) as rsrp,
            tc.tile_pool(name="ocp", bufs=2) as ocp,
            tc.tile_pool(name="x2", bufs=4) as x2pool,
            tc.tile_pool(name="o3", bufs=4) as o3pool,
        ):
            cx.xin, cx.s1ps, cx.spsum = xin, s1ps, spsum
            cx.apsum, cx.oppj = apsum, oppj
            cx.att, cx.rsp, cx.ropetmp, cx.ocp = att, rsp, ropetmp, ocp
            cx.rsrp = rsrp
            cx.x2pool, cx.o3pool = x2pool, o3pool
            # x-feed rings (gpsimd excluded: it carries weights + oT + x2)
            cx.dma3 = [nc.sync, nc.scalar]
            cx.outdma = [nc.sync, nc.scalar]

            # ---- warmup collective: absorbs the one-time rendezvous cost
            # (~35us) of the first real AllToAll, concurrent with stage 1 ----
            cx.wq_sb = consts.tile([128, p["KT"], p["HD"]], BF16, tag="wq")
            wqr = wq[:, :].rearrange("p (kt n) -> p kt n", kt=p["KT"])
            nc.gpsimd.dma_start(out=cx.wq_sb[:, 0:4, :], in_=wqr[:, 0:4, :])
            nc.gpsimd.dma_start(out=cx.wq_sb[:, 4:, :], in_=wqr[:, 4:, :])
            warm_in = dram.tile([n_cores, 128], BF16, tag="warmi", name="warm_in")
            warm_out = dram.tile([n_cores, 128], BF16, tag="warm", name="warm_out")
            nc.gpsimd.dma_start(out=warm_in, in_=maskc[0:n_cores, :])
            nc.gpsimd.collective_compute(
                "AllToAll", mybir.AluOpType.bypass,
                replica_groups=[list(range(n_cores))],
                ins=[warm_in[:, :].opt()],
                outs=[warm_out[:, :].opt()],
            )

            # ---- startup-critical loads: wq + cos/sin on gpsimd; wk/wv are
            # emitted after the first two x chunks enter the sync/scalar
            # rings (batch 0 runs Q-before-K so Q starts as soon as wq+x0
            # land) ----
            cx.wk_sb = consts.tile([128, p["KT"], p["HD"]], BF16, tag="wk")
            cx.wv_sb = consts.tile([128, p["KT"], p["HD"]], BF16, tag="wv")
            cx.cos_sb = consts.tile([128, t], BF16, tag="cos")
            cx.sin_sb = consts.tile([128, t], BF16, tag="sin")
            nc.gpsimd.dma_start(out=cx.cos_sb, in_=cosT[:, :])
            nc.gpsimd.dma_start(out=cx.sin_sb, in_=sinT[:, :])
            cx.bq_sb = consts.tile([128, HL], F32, tag="bq")
            cx.bk_sb = consts.tile([128, HL], F32, tag="bk")
            cx.bvh_sb = consts.tile([128, HL], F32, tag="bvh")
            cx.maskU_sb = consts.tile([128, 128], BF16, tag="mask")

            # ---- rolling 2-batch QKV buffers ----
            cx.qkvpool = qkvres
            cx.qkv = {}

            cx.a2a_ins = {}
            cx.a2a_outs = {}
            for bb in range(b):
                for hm in range(HL):
                    cx.a2a_ins[(bb, hm)] = dram.tile(
                        [n_cores * d, seg], BF16,
                        tag=f"a2a_in{bb}_{hm}", name=f"a2a_in{bb}_{hm}")
                    cx.a2a_outs[(bb, hm)] = dram.tile(
                        [n_cores * d, seg], BF16,
                        tag=f"a2a_out{bb}_{hm}", name=f"a2a_out{bb}_{hm}")
            cx.x2 = {}
            cx.op_tiles = {}
            cx.pending_gathers = []

            # ---- prologue: stage 1 of batch 0 (straight emission, Q before
            # K so Q starts on wq+x0 alone) ----
            _alloc_qkv(cx, 0)
            s1_b0 = [_s1_gen(nc, cx, 0, rc) for rc in range(n_rc)]
            # first x chunk enters the rings, then wk/wv, then the second
            next(s1_b0[0])
            nc.sync.dma_start(out=cx.wk_sb, in_=wk[:, :].rearrange("p (kt n) -> p kt n", kt=p["KT"]))
            nc.scalar.dma_start(out=cx.wv_sb, in_=wv[:, :].rearrange("p (kt n) -> p kt n", kt=p["KT"]))
            next(s1_b0[1])
            nc.sync.dma_start(out=cx.bq_sb, in_=bq[:, :])
            nc.sync.dma_start(out=cx.bk_sb, in_=bk[:, :])
            nc.sync.dma_start(out=cx.bvh_sb, in_=bvh[:, :])
            nc.sync.dma_start(out=cx.maskU_sb, in_=maskc[:, :])
            # deferred big loads on gpsimd (its oT-stream traffic only starts
            # mid-window-0)
            cx.wo_sb = []
            for nn2 in range(4):
                wt = consts.tile([128, p["KT"], 512], BF16, tag=f"wo{nn2}")
                nc.gpsimd.dma_start(
                    out=wt, in_=cx.wo[nn2 * 128:(nn2 + 1) * 128, :].rearrange(
                        "p (kt n) -> p kt n", kt=p["KT"]))
                cx.wo_sb.append(wt)
            for g in s1_b0:
                _drain(g)

            # ---- batch-pipelined main loop ----
            for bb in range(b):
                if bb + 1 < b:
                    _alloc_qkv(cx, bb + 1)
                _window(nc, cx, bb, b)

            # ---- tail: last batch's outproj. Even-kt halves of the m=0
            # units depend only on head 0's collective and cover the final
            # collective's flight; the rest completes after it lands. PSUM
            # comes from the (now idle) attention pool so back-to-back units
            # double-buffer ----
            bl = b - 1
            _flush_gathers(nc, cx)
            for nn in range(4):
                _drain(_outproj_gen(nc, cx, bl, nn, 0, half=0, pool=apsum))
            for nn in range(4):
                _drain(_outproj_gen(nc, cx, bl, nn, 1,
                                    pool=apsum if nn % 2 == 0 else None,
                                    stage_pool=ocp))
            for nn in range(4):
                _drain(_outproj_gen(nc, cx, bl, nn, 0, half=1, pool=apsum))

    nc.compile()
    return nc


def _host_prep(x_norm, Wqkv, bqkv, Wout, bout, b, t, c, h, d, n_cores):
    """Build per-core input maps (numpy, bf16)."""
    HL = h // n_cores
    R = b * t
    perm = np.concatenate([np.arange(0, d, 2), np.arange(1, d, 2)])  # deinterleave

    RC = 512
    XT = x_norm.reshape(R, c).T.astype(NPBF16)          # [C, R]
    # pre-tile: [R//RC, 128, KT*RC] so each row-chunk DMA is contiguous
    KT = c // 128
    xB = np.ascontiguousarray(
        XT.reshape(KT, 128, R // RC, RC).transpose(2, 1, 0, 3)
    ).reshape((R // RC) * 128, KT * RC)
    inv_freq = 1.0 / (ROPE_BASE ** (np.arange(0, d, 2, dtype=np.float64) / d))
    ang = np.arange(t, dtype=np.float64)[None, :] * inv_freq[:, None]  # [d/2, t]
    cosT = np.concatenate([np.cos(ang), np.cos(ang)], axis=0).astype(NPBF16)
    # sign-folded sin, laid out to match the *input* partition range of each
    # RoPE mul: rows 0-63 = +sin (multiplies x0 into rt[64:128]),
    # rows 64-127 = -sin (multiplies x1 into rt[0:64])
    sinT = np.concatenate([np.sin(ang), -np.sin(ang)], axis=0).astype(NPBF16)
    # upper-triangular (incl diagonal) 0/1 mask for the transposed P layout
    maskc = np.triu(np.ones((128, 128), dtype=np.float32)).astype(NPBF16)
    wo_bf = Wout.astype(NPBF16)                          # [C, C]
    wo_b = np.ascontiguousarray(
        wo_bf.reshape(KT, 128, c // 512, 512).transpose(2, 1, 0, 3)
    ).reshape((c // 512) * 128, KT * 512)

    in_maps = []
    for i in range(n_cores):
        cols_q = np.concatenate([i * HL * d + hh * d + perm for hh in range(HL)])
        cols_k = cols_q + h * d
        cols_v = np.concatenate([2 * h * d + i * HL * d + hh * d + np.arange(d)
                                 for hh in range(HL)])
        # pretile [C, HD] -> [128, KT*HD] (row p = concat over kt of W[kt*128+p, :])
        def _wtile(w):
            return np.ascontiguousarray(
                w.reshape(KT, 128, HL * d).transpose(1, 0, 2).reshape(128, KT * HL * d))
        wq_i = _wtile(Wqkv[:, cols_q].astype(NPBF16))
        wk_i = _wtile(Wqkv[:, cols_k].astype(NPBF16))
        wv_i = _wtile(Wqkv[:, cols_v].astype(NPBF16))
        bq_i = np.stack([bqkv[i * HL * d + hh * d + perm] for hh in range(HL)],
                        axis=1).astype(np.float32)
        bk_i = np.stack([bqkv[h * d + i * HL * d + hh * d + perm] for hh in range(HL)],
                        axis=1).astype(np.float32)
        # V bias folded into the attention epilogue (per-partition over d)
        bv_nat = bqkv[2 * h * d + i * HL * d:2 * h * d + (i + 1) * HL * d]
        bvh_i = np.stack([bv_nat[hh * d:(hh + 1) * d] for hh in range(HL)],
                         axis=1).astype(np.float32)
        in_maps.append({
            "xB": xB, "wq": wq_i, "wk": wk_i, "wv": wv_i,
            "bq": np.ascontiguousarray(bq_i), "bk": np.ascontiguousarray(bk_i),
            "bvh": np.ascontiguousarray(bvh_i),
            "wo": wo_b, "cosT": cosT, "sinT": sinT, "maskc": maskc,
        })
    return in_maps


def _gather(parts, b, t, c, n_cores):
    """Core j's out rows are, for each batch bb, global rows
    [bb*t + j*seg, bb*t + (j+1)*seg) with seg = t // n_cores."""
    seg = t // n_cores
    R = b * t
    full = np.empty((R, c), dtype=np.float32)
    for j in range(n_cores):
        for bb in range(b):
            full[bb * t + j * seg: bb * t + (j + 1) * seg] = \
                parts[j][bb * seg:(bb + 1) * seg]
    return full.reshape(b, t, c)


_NC_CACHE = {}


def kernel(x_norm, Wqkv, bqkv, Wout, bout):
    b, t, c = x_norm.shape
    h = 16
    d = c // h
    key = (b, t, c)
    if key not in _NC_CACHE:
        _NC_CACHE[key] = build_nc(b, t, c, h, d, N_CORES)
    nc = _NC_CACHE[key]
    in_maps = _host_prep(np.asarray(x_norm, dtype=np.float32),
                         np.asarray(Wqkv, dtype=np.float32),
                         np.asarray(bqkv, dtype=np.float32),
                         np.asarray(Wout, dtype=np.float32),
                         np.asarray(bout, dtype=np.float32),
                         b, t, c, h, d, N_CORES)
    res = run_bass_kernel_spmd(nc, in_maps, core_ids=list(range(N_CORES)))
    parts = [np.asarray(res.results[i]["out"], dtype=np.float32) for i in range(N_CORES)]
    full = _gather(parts, b, t, c, N_CORES)
    # bout is folded in host-side (the device-side drain is then a plain
    # scalar-engine copy, off the vector queue)
    full += np.asarray(bout, dtype=np.float32)[None, None, :]
    return full


# revision 62
# speedup vs baseline: 1.5665x; 1.5665x over previous
"""Trainium2 distributed kernel for nn_Attention (dense transformer attention block).

Strategy (8 NeuronCores, tensor-parallel over heads, batch-pipelined,
fine-grained weave):
  - Host pre-transposes x_norm -> X^T [C, B*T] (bf16) and slices Wqkv columns
    per core (2 heads/core, deinterleaved RoPE feature order). RoPE sin/cos
    tables precomputed host-side (sin is sign-folded for full-128 ops).
  - Emission is piece-granular: stage-1 (next batch) and out-projection
    (previous batch) units are generators whose ~0.5us pieces are woven
    between every attention S-matmul and its dependent PV-matmul, so the
    exp (ACT) latency never stalls the in-order tensor queue.
  - stage1(bb): per 256-row chunk: Q^T/K^T (head-major, D on partitions,
    packed q|k PSUM) + V natural (packed PSUM). Bias epilogues on gpsimd,
    V copy on vector, RoPE on vector (scalar engine is reserved for exp).
  - attention: S^T flash form without max-subtraction: S^T tile -> exp (ACT,
    scaled) -> P^T bf16 -> V^T@P^T accumulate (PSUM) + rowsums (vector, bf16)
    -> gpsimd partition allreduce -> reciprocal -> normalize into a per-chunk
    staging tile streamed straight to the a2a dram tile.
  - Per-(batch, head) AllToAll (0.5 MiB bf16) issued as soon as that head
    finishes; the per-head x2 gather DMA is issued immediately after the
    collective so out-projection inputs land long before they are needed.
  - Wout is resident in SBUF (loaded once, 4 tiles spread over 4 rings).
  - Tail: out-projection of the last batch is split even/odd-kt: even kts
    depend only on head 0's collective and cover the final collective's
    flight; odd kts complete the contraction once it lands.
  - Startup: first x chunk + wq land on separate rings before anything else;
    cos/sin/bo and Wout loads are deferred behind them.
  - Host reassembles the per-(core, batch) row pieces -> [B, T, C] fp32.
"""

import numpy as np
import ml_dtypes

import concourse.bass as bass
import concourse.bass_isa as bass_isa
import concourse.mybir as mybir
import concourse.tile as tile
from concourse import bacc
from concourse.bass_utils import run_bass_kernel_spmd


N_CORES = 8
B, T, C = 4, 2048, 2048
H, D = 16, 128
ROPE_BASE = 10000.0

BF16 = mybir.dt.bfloat16
F32 = mybir.dt.float32
NPBF16 = ml_dtypes.bfloat16


class Ctx:
    """Bag of tiles/pools/params shared by the emission helpers."""
    pass


def _alloc_qkv(cx, bb):
    """Rolling per-batch QKV tiles (pool bufs=2 -> 2 batches in flight)."""
    p = cx.p
    cx.qkv[bb] = (
        cx.qkvpool.tile([128, p["HL"], p["t"]], BF16, tag="qT", name=f"qT{bb}"),
        cx.qkvpool.tile([128, p["HL"], p["t"]], BF16, tag="kT", name=f"kT{bb}"),
        cx.qkvpool.tile([128, p["t"] // 128, p["HD"]], BF16, tag="v", name=f"v{bb}"),
    )


def _s1_gen(nc, cx, bb, rc):
    """Stage-1 unit (512 rows) as a generator of ~0.5us pieces. Target-outer:
    each of Q0/Q1/K0/K1 accumulates its own 1-bank PSUM tile over all kt
    (512-col matmuls, half the instruction count of the 256-row form), then
    V in two packed psum tiles. Q targets run first so batch 0 can start on
    wq + x0 alone."""
    p = cx.p
    RC, KT, HL = p["RC"], p["KT"], p["HL"]
    t = p["t"]
    r0g = bb * t + rc * RC
    t0 = rc * RC
    qT_t, kT_t, v_t = cx.qkv[bb]
    rcg = r0g // RC
    xt = cx.xin.tile([128, KT, RC], BF16, tag="xt")
    # each chunk is fetched as two parallel halves (one per ring) to halve
    # its arrival latency; batch 0's unit 2 rides the gpsimd ring so three
    # rings feed the unwoven prologue
    xsrc = cx.xB[rcg * 128:(rcg + 1) * 128, :].rearrange(
        "p (kt r) -> p kt r", kt=KT)
    hk = KT // 2
    rings = ((nc.gpsimd, nc.gpsimd) if (bb == 0 and rc == 2)
             else (cx.dma3[0], cx.dma3[1]))
    if bb == 0 and rc == 0:
        # quarter-granular so the first Q matmul starts on kt 0-3 alone
        qk_ = KT // 4
        for qi in range(4):
            rings[qi % 2].dma_start(out=xt[:, qi * qk_:(qi + 1) * qk_, :],
                                    in_=xsrc[:, qi * qk_:(qi + 1) * qk_, :])
    else:
        rings[0].dma_start(out=xt[:, 0:hk, :], in_=xsrc[:, 0:hk, :])
        rings[1].dma_start(out=xt[:, hk:KT, :], in_=xsrc[:, hk:KT, :])
    yield
    for which, w_sb, dstt, bias_sb in (
            (0, cx.wq_sb, qT_t, cx.bq_sb), (1, cx.wk_sb, kT_t, cx.bk_sb)):
        for hm in range(HL):
            ps = cx.s1ps.tile([128, RC], F32, tag="s1ps",
                              name=f"ps{bb}_{rc}_{which}_{hm}")
            for kt in range(KT):
                nc.tensor.matmul(ps, lhsT=w_sb[:, kt, hm * 128:(hm + 1) * 128],
                                 rhs=xt[:, kt, :],
                                 start=(kt == 0), stop=(kt == KT - 1))
                if kt % 2 == 1:
                    yield
            nc.scalar.activation(out=dstt[:, hm, t0:t0 + RC], in_=ps,
                                 func=mybir.ActivationFunctionType.Identity,
                                 bias=bias_sb[:, hm:hm + 1], scale=1.0)
    # ---- V natural: two psum tiles, each packing two 128-row blocks ----
    for vg in range(RC // 256):
        psv = cx.s1ps.tile([128, 2 * p["HD"]], F32, tag="s1ps",
                           name=f"psv{bb}_{rc}_{vg}")
        for kt in range(KT):
            for rr in range(2):
                rs = 2 * vg + rr
                nc.tensor.matmul(psv[:, rr * p["HD"]:(rr + 1) * p["HD"]],
                                 lhsT=xt[:, kt, rs * 128:(rs + 1) * 128],
                                 rhs=cx.wv_sb[:, kt, :],
                                 start=(kt == 0 and rr == 0), stop=(kt == KT - 1))
            if kt % 2 == 1:
                yield
        for rr in range(2):
            rt_ = (t0 // 128) + 2 * vg + rr
            nc.scalar.activation(out=v_t[:, rt_, :],
                                 in_=psv[:, rr * p["HD"]:(rr + 1) * p["HD"]],
                                 func=mybir.ActivationFunctionType.Copy, scale=1.0)
        yield
    # ---- RoPE in place on this 512-row span (both heads, q and k) ----
    # x0' = x0*cos - x1*sin ; x1' = x1*cos + x0*sin
    # (sin sign-folded host-side: rows 0-63 = +sin, rows 64-127 = -sin)
    for hm in range(HL):
        for res in (qT_t, kT_t):
            x = res[:, hm, t0:t0 + RC]
            rt = cx.ropetmp.tile([128, RC], BF16, tag="rt")
            nc.vector.tensor_mul(rt[0:64, :], x[64:128, :], cx.sin_sb[64:128, t0:t0 + RC])
            yield
            nc.vector.tensor_mul(rt[64:128, :], x[0:64, :], cx.sin_sb[0:64, t0:t0 + RC])
            nc.vector.tensor_mul(x, x, cx.cos_sb[:, t0:t0 + RC])
            nc.vector.tensor_add(x, x, rt)
        yield


S1_PIECES = 57   # yields per _s1_gen: 1 xt + 32 qk + 18 v + 6 rope
OP_PIECES = 3    # yields per full _outproj_gen (16 kts, yield every 4th, not last)


def _attn_gen(nc, cx, bb, hm, c):
    """Attention unit generator: yields after S(+exp) and after PV per jt."""
    p = cx.p
    t, d = p["t"], p["d"]
    SCALE = p["SCALE"]
    tq0 = c * 512
    jt_max = 4 * (c + 1)
    qT_t, kT_t, v_t = cx.qkv[bb]
    qT_h = qT_t[:, hm, :]
    kT_h = kT_t[:, hm, :]
    psum_o = cx.apsum.tile([128, 512], F32, tag="po", name=f"po{bb}_{hm}_{c}")
    # bf16 rowsum accumulation: DVE runs 2x mode; allreduce upcasts to f32
    rs_d = cx.rsp.tile([128, 512], BF16, tag="rsd", name=f"rsd{bb}_{hm}_{c}")
    for jt in range(jt_max):
        off = max(0, jt * 128 - tq0)
        st = cx.spsum.tile([128, 512], F32, tag="st", name=f"st{bb}_{hm}_{c}_{jt}")
        nc.tensor.matmul(st[:, off:512],
                         lhsT=kT_h[:, jt * 128:(jt + 1) * 128],
                         rhs=qT_h[:, tq0 + off:tq0 + 512],
                         start=True, stop=True)
        pT = cx.att.tile([128, 512], BF16, tag="pT", name=f"pT{bb}_{hm}_{c}_{jt}")
        nc.scalar.activation(out=pT[:, off:512], in_=st[:, off:512],
                             func=mybir.ActivationFunctionType.Exp, scale=SCALE)
        if jt * 128 >= tq0:
            nc.vector.tensor_mul(pT[:, off:off + 128], pT[:, off:off + 128],
                                 cx.maskU_sb)
        yield  # fillers injected here hide the exp latency
        nc.tensor.matmul(psum_o[:, off:512],
                         lhsT=v_t[:, jt, hm * d:(hm + 1) * d],
                         rhs=pT[:, off:512],
                         start=(jt == 0), stop=(jt == jt_max - 1))
        if jt == 0:
            nc.vector.tensor_copy(rs_d, pT)
        else:
            nc.vector.tensor_add(rs_d[:, off:512], rs_d[:, off:512], pT[:, off:512])
        yield
    rs_red = cx.rsrp.tile([128, 512], F32, tag="rsr", name=f"rsr{bb}_{hm}_{c}")
    nc.gpsimd.partition_all_reduce(rs_red, rs_d, 128, bass_isa.ReduceOp.add)
    nc.vector.reciprocal_approx_fast(out=rs_red, in_=rs_red)
    oc = cx.ocp.tile([128, 512], BF16, tag="oc", name=f"oc{bb}_{hm}_{c}")
    nc.vector.tensor_mul(oc, psum_o, rs_red)
    nc.vector.tensor_scalar_add(oc, oc, cx.bvh_sb[:, hm:hm + 1])
    # stream this chunk's two slot-columns into the a2a dram tile now, so the
    # payload write is off the collective's critical path. These moves are
    # 512B-packet-bound (~10us each), so round-robin them over three rings.
    eng = (nc.gpsimd, nc.sync, nc.scalar)[(hm * 4 + c) % 3]
    eng.dma_start(
        out=cx.a2a_ins[(bb, hm)][2 * c * 128:(2 * c + 2) * 128, :].rearrange(
            "(sl dd) r -> dd sl r", dd=p["d"]),
        in_=oc.rearrange("p (sl r) -> p sl r", sl=2))


def _issue_a2a(nc, cx, bb, hm):
    """Issue the AllToAll for (bb, hm). The dependent x2 gather is queued and
    flushed at the NEXT collective boundary: a gather issued immediately
    would head-of-line-block its DMA rings for the whole collective flight,
    delaying the next head's payload streams."""
    p = cx.p
    n_cores = N_CORES
    # any pending gather's collective has had a full head's compute to land
    _flush_gathers(nc, cx)
    nc.gpsimd.collective_compute(
        "AllToAll", mybir.AluOpType.bypass,
        replica_groups=[list(range(n_cores))],
        ins=[cx.a2a_ins[(bb, hm)][:, :].opt()],
        outs=[cx.a2a_outs[(bb, hm)][:, :].opt()],
    )
    cx.pending_gathers.append((bb, hm))


def _flush_gathers(nc, cx):
    """Emit the x2 gather DMAs for all pending collectives (their flights
    have ended, or nothing time-critical sits behind them on the ring).
    Each gather is split across two rings: it moves 512B packets, so one
    ring alone takes ~15us."""
    p = cx.p
    seg, KT, HL = p["t"] // N_CORES, p["KT"], p["HL"]
    for bb, hm in cx.pending_gathers:
        x2h = cx.x2pool.tile([128, KT // HL, seg], BF16, tag="x2",
                             name=f"x2_{bb}_{hm}")
        src = cx.a2a_outs[(bb, hm)][:, :].rearrange("(sl p) r -> p sl r", p=128)
        nc.gpsimd.dma_start(out=x2h[:, 0:3, :], in_=src[:, 0:3, :])
        nc.sync.dma_start(out=x2h[:, 3:6, :], in_=src[:, 3:6, :])
        nc.scalar.dma_start(out=x2h[:, 6:8, :], in_=src[:, 6:8, :])
        cx.x2[(bb, hm)] = x2h
    cx.pending_gathers = []


def _outproj_gen(nc, cx, bb, nn, m, half=None, pool=None, stage_pool=None):
    """Out-projection unit generator: one (batch, 512-outcol, 128-row) chunk.

    half=None: full 16-kt contraction. half=0: even kts (head-0 channels,
    depends only on that head's collective), parked in SBUF. half=1: odd
    kts, added to the parked half, then written out."""
    p = cx.p
    KT, HL = p["KT"], p["HL"]
    seg = p["t"] // N_CORES
    kts = (list(range(0, KT, 2)) + list(range(1, KT, 2))
           if half is None else list(range(half, KT, 2)))
    # tag matches the host pool's resident tag so no extra bank is reserved
    ps3 = (pool or cx.oppj).tile([128, 512], F32,
                                 tag="po" if pool is not None else "pj",
                                 name=f"ps3{bb}_{nn}_{m}_{half}")
    for i, kt in enumerate(kts):
        x2h = cx.x2[(bb, kt % HL)]
        nc.tensor.matmul(ps3, lhsT=x2h[:, kt // HL, m * 128:(m + 1) * 128],
                         rhs=cx.wo_sb[nn][:, kt, :],
                         start=(i == 0), stop=(i == len(kts) - 1))
        if i % 4 == 3 and i != len(kts) - 1:
            yield
    # bout is added host-side, so the PSUM drain is a plain copy that can run
    # on scalar (keeping the outproj-PSUM recycle off the vector queue)
    if half == 0:
        o3 = cx.o3pool.tile([128, 512], BF16, tag="o3", name=f"o3{bb}_{nn}_{m}")
        cx.op_tiles[(bb, nn, m, "o3")] = o3
        nc.scalar.activation(out=o3, in_=ps3,
                             func=mybir.ActivationFunctionType.Copy, scale=1.0)
        return
    if half == 1:
        o3 = cx.op_tiles[(bb, nn, m, "o3")]
        nc.vector.tensor_add(o3, o3, ps3)
    else:
        o3 = (stage_pool or cx.o3pool).tile(
            [128, 512], BF16, tag="oc" if stage_pool is not None else "o3",
            name=f"o3{bb}_{nn}_{m}")
        nc.scalar.activation(out=o3, in_=ps3,
                             func=mybir.ActivationFunctionType.Copy, scale=1.0)
    cx.outdma[(bb * 2 + m) % 2].dma_start(
        out=cx.out[bb * seg + m * 128:bb * seg + (m + 1) * 128,
                   nn * 512:(nn + 1) * 512], in_=o3)


def _drain(gen):
    for _ in gen:
        pass


def _window(nc, cx, bb, b):
    """Emit one pipeline window: attention(bb) woven with stage1(bb+1) and
    outproj fillers; collectives at head boundaries."""
    p = cx.p
    HL = p["HL"]
    # flush the previous window's x2 gathers: those collectives have landed,
    # so the gathers won't block the ring
    _flush_gathers(nc, cx)
    # attention units: big chunk (c=3) first so each head's last-finished
    # chunk is the small one, shortening the chain into its collective
    attn_units = [(hm, cc) for hm in range(HL) for cc in (3, 2, 1, 0)]
    n_slots = sum(2 * 4 * (cc + 1) + 1 for _, cc in attn_units)

    s1_gens = ([_s1_gen(nc, cx, bb + 1, rc) for rc in range(p["t"] // p["RC"])]
               if bb + 1 < b else [])
    # outproj filler assignment: window 1 takes batch 0; the second-to-last
    # window is left without outproj so the LAST window (which has no stage-1
    # filler) gets two batches' worth of filler matmuls to hide exp latency
    if bb == 0 or bb == b - 2:
        op_gens = []
    elif bb < b - 2:
        op_gens = [_outproj_gen(nc, cx, bb - 1, nn, m)
                   for nn in range(4) for m in range(2)]
    else:
        op_gens = [_outproj_gen(nc, cx, bbx, nn, m)
                   for bbx in (bb - 2, bb - 1)
                   for nn in range(4) for m in range(2)]
    prim = list(s1_gens)
    sec = list(op_gens)

    def pull(q):
        while q:
            try:
                next(q[0])
                return True
            except StopIteration:
                q.pop(0)
        return False

    # Front-load stage-1: exhaust it by ~70% of the window so the last
    # units' epilogues/RoPE have slack before the next window's attention
    # consumes qT/kT/v. Outproj is spread evenly (after a short warmup so
    # its first matmuls never head the tensor queue before inputs land).
    prim_rate = S1_PIECES * len(s1_gens) / (0.6 * n_slots)
    sec_rate = OP_PIECES * len(op_gens) / max(1, n_slots - 24)
    state = {"prim_debt": 0.0, "sec_debt": -24.0 * sec_rate}

    def pace():
        state["prim_debt"] += prim_rate
        state["sec_debt"] += sec_rate
        while state["prim_debt"] >= 1.0 and pull(prim):
            state["prim_debt"] -= 1.0
        if not prim:
            state["prim_debt"] = 0.0
        while state["sec_debt"] >= 1.0 and pull(sec):
            state["sec_debt"] -= 1.0
        if not sec:
            state["sec_debt"] = 0.0

    def drive(units, interleave=False):
        gens = [_attn_gen(nc, cx, bb, hm, cc) for hm, cc in units]
        if interleave:
            live = list(gens)
            while live:
                nxt = []
                for g in live:
                    try:
                        next(g)
                        pace()
                        nxt.append(g)
                    except StopIteration:
                        pass
                live = nxt
        else:
            for g in gens:
                while True:
                    try:
                        next(g)
                    except StopIteration:
                        break
                    pace()

    for ui, (hm, cc) in enumerate(attn_units):
        drive([(hm, cc)])
        if cc == 0:
            _issue_a2a(nc, cx, bb, hm)
        elif ui == 4:
            # head 0's collective has had a full unit's compute to land
            _flush_gathers(nc, cx)
    # drain leftover fillers before the next window
    for gg in prim + sec:
        _drain(gg)


def build_nc(b=B, t=T, c=C, h=H, d=D, n_cores=N_CORES):
    HL = h // n_cores          # heads per core
    R = b * t                  # total rows
    RS = R // n_cores          # rows per core overall (output)
    RC = 512                   # row-chunk for stage 1
    p = dict(b=b, t=t, c=c, h=h, d=d, HL=HL, R=R, RS=RS, RC=RC,
             KT=c // 128, HD=HL * d, SCALE=1.0 / float(np.sqrt(d)))
    seg = t // n_cores
    n_rc = t // RC             # stage-1 units per batch

    nc = bacc.Bacc(None, target_bir_lowering=False, debug=False,
                   num_devices=n_cores)

    cx = Ctx()
    cx.p = p
    cx.xB = nc.declare_dram_parameter("xB", [(R // RC) * 128, (c // 128) * RC], BF16, isOutput=False)
    # weights host-pretiled to [128, KT*HD] so each load is 128 contiguous
    # 8KB descriptors instead of 2048 x 512B packets
    wq = nc.declare_dram_parameter("wq", [128, p["KT"] * p["HD"]], BF16, isOutput=False)
    wk = nc.declare_dram_parameter("wk", [128, p["KT"] * p["HD"]], BF16, isOutput=False)
    wv = nc.declare_dram_parameter("wv", [128, p["KT"] * p["HD"]], BF16, isOutput=False)
    bq = nc.declare_dram_parameter("bq", [128, HL], F32, isOutput=False)
    bk = nc.declare_dram_parameter("bk", [128, HL], F32, isOutput=False)
    bvh = nc.declare_dram_parameter("bvh", [128, HL], F32, isOutput=False)
    cx.wo = nc.declare_dram_parameter("wo", [(c // 512) * 128, (c // 128) * 512], BF16, isOutput=False)
    cosT = nc.declare_dram_parameter("cosT", [128, t], BF16, isOutput=False)
    sinT = nc.declare_dram_parameter("sinT", [128, t], BF16, isOutput=False)
    maskc = nc.declare_dram_parameter("maskc", [128, 128], BF16, isOutput=False)
    cx.out = nc.declare_dram_parameter("out", [RS, c], BF16, isOutput=True)

    with tile.TileContext(nc) as tc:
        with (
            tc.tile_pool(name="consts", bufs=1) as consts,
            tc.tile_pool(name="qkvres", bufs=2) as qkvres,
            tc.tile_pool(name="dram", bufs=1, space="DRAM") as dram,
            tc.tile_pool(name="xin", bufs=2) as xin,
            tc.tile_pool(name="ropet", bufs=1) as ropetmp,
            tc.tile_pool(name="s1ps", bufs=3, space="PSUM") as s1ps,
            tc.tile_pool(name="spsum", bufs=2, space="PSUM") as spsum,
            tc.tile_pool(name="apsum", bufs=2, space="PSUM") as apsum,
            tc.tile_pool(name="oppj", bufs=1, space="PSUM") as oppj,
            tc.tile_pool(name="att", bufs=3) as att,
            tc.tile_pool(name="rsp", bufs=2) as rsp,
            tc.tile_pool(name="rsr", bufs=1) as rsrp,
            tc.tile_pool(name="ocp", bufs=2) as ocp,
            tc.tile_pool(name="x2", bufs=4) as x2pool,
            tc.tile_pool(name="o3", bufs=4) as o3pool,
        ):
            cx.xin, cx.s1ps, cx.spsum = xin, s1ps, spsum
            cx.apsum, cx.oppj = apsum, oppj
            cx.att, cx.rsp, cx.ropetmp, cx.ocp = att, rsp, ropetmp, ocp
            cx.rsrp = rsrp
            cx.x2pool, cx.o3pool = x2pool, o3pool
            # x-feed rings (gpsimd excluded: it carries weights + oT + x2)
            cx.dma3 = [nc.sync, nc.scalar]
            cx.outdma = [nc.sync, nc.scalar]

            # ---- warmup collective: absorbs the one-time rendezvous cost
            # (~35us) of the first real AllToAll, concurrent with stage 1 ----
            cx.wq_sb = consts.tile([128, p["KT"], p["HD"]], BF16, tag="wq")
            wqr = wq[:, :].rearrange("p (kt n) -> p kt n", kt=p["KT"])
            nc.gpsimd.dma_start(out=cx.wq_sb[:, 0:4, :], in_=wqr[:, 0:4, :])
            nc.gpsimd.dma_start(out=cx.wq_sb[:, 4:, :], in_=wqr[:, 4:, :])
            warm_in = dram.tile([n_cores, 128], BF16, tag="warmi", name="warm_in")
            warm_out = dram.tile([n_cores, 128], BF16, tag="warm", name="warm_out")
            nc.gpsimd.dma_start(out=warm_in, in_=maskc[0:n_cores, :])
            nc.gpsimd.collective_compute(
                "AllToAll", mybir.AluOpType.bypass,
                replica_groups=[list(range(n_cores))],
                ins=[warm_in[:, :].opt()],
                outs=[warm_out[:, :].opt()],
            )

            # ---- startup-critical loads: wq + cos/sin on gpsimd; wk/wv are
            # emitted after the first two x chunks enter the sync/scalar
            # rings (batch 0 runs Q-before-K so Q starts as soon as wq+x0
            # land) ----
            cx.wk_sb = consts.tile([128, p["KT"], p["HD"]], BF16, tag="wk")
            cx.wv_sb = consts.tile([128, p["KT"], p["HD"]], BF16, tag="wv")
            cx.cos_sb = consts.tile([128, t], BF16, tag="cos")
            cx.sin_sb = consts.tile([128, t], BF16, tag="sin")
            nc.gpsimd.dma_start(out=cx.cos_sb, in_=cosT[:, :])
            nc.gpsimd.dma_start(out=cx.sin_sb, in_=sinT[:, :])
            cx.bq_sb = consts.tile([128, HL], F32, tag="bq")
            cx.bk_sb = consts.tile([128, HL], F32, tag="bk")
            cx.bvh_sb = consts.tile([128, HL], F32, tag="bvh")
            cx.maskU_sb = consts.tile([128, 128], BF16, tag="mask")

            # ---- rolling 2-batch QKV buffers ----
            cx.qkvpool = qkvres
            cx.qkv = {}

            cx.a2a_ins = {}
            cx.a2a_outs = {}
            for bb in range(b):
                for hm in range(HL):
                    cx.a2a_ins[(bb, hm)] = dram.tile(
                        [n_cores * d, seg], BF16,
                        tag=f"a2a_in{bb}_{hm}", name=f"a2a_in{bb}_{hm}")
                    cx.a2a_outs[(bb, hm)] = dram.tile(
                        [n_cores * d, seg], BF16,
                        tag=f"a2a_out{bb}_{hm}", name=f"a2a_out{bb}_{hm}")
            cx.x2 = {}
            cx.op_tiles = {}
            cx.pending_gathers = []

            # ---- prologue: stage 1 of batch 0 (straight emission, Q before
            # K so Q starts on wq+x0 alone) ----
            _alloc_qkv(cx, 0)
            s1_b0 = [_s1_gen(nc, cx, 0, rc) for rc in range(n_rc)]
            # first x chunk enters the rings, then wk/wv, then the second
            next(s1_b0[0])
            nc.sync.dma_start(out=cx.wk_sb, in_=wk[:, :].rearrange("p (kt n) -> p kt n", kt=p["KT"]))
            nc.scalar.dma_start(out=cx.wv_sb, in_=wv[:, :].rearrange("p (kt n) -> p kt n", kt=p["KT"]))
            next(s1_b0[1])
            nc.sync.dma_start(out=cx.bq_sb, in_=bq[:, :])
            nc.sync.dma_start(out=cx.bk_sb, in_=bk[:, :])
            nc.sync.dma_start(out=cx.bvh_sb, in_=bvh[:, :])
            nc.sync.dma_start(out=cx.maskU_sb, in_=maskc[:, :])
            # deferred big loads on gpsimd (its oT-stream traffic only starts
            # mid-window-0)
            cx.wo_sb = []
            for nn2 in range(4):
                wt = consts.tile([128, p["KT"], 512], BF16, tag=f"wo{nn2}")
                nc.gpsimd.dma_start(
                    out=wt, in_=cx.wo[nn2 * 128:(nn2 + 1) * 128, :].rearrange(
                        "p (kt n) -> p kt n", kt=p["KT"]))
                cx.wo_sb.append(wt)
            for g in s1_b0:
                _drain(g)

            # ---- batch-pipelined main loop ----
            for bb in range(b):
                if bb + 1 < b:
                    _alloc_qkv(cx, bb + 1)
                _window(nc, cx, bb, b)

            # ---- tail: last batch's outproj. Even-kt halves of the m=0
            # units depend only on head 0's collective and cover the final
            # collective's flight; the rest completes after it lands. PSUM
            # comes from the (now idle) attention pool so back-to-back units
            # double-buffer ----
            bl = b - 1
            _flush_gathers(nc, cx)
            for nn in range(4):
                _drain(_outproj_gen(nc, cx, bl, nn, 0, half=0, pool=apsum))
            for nn in range(4):
                _drain(_outproj_gen(nc, cx, bl, nn, 1,
                                    pool=apsum if nn % 2 == 0 else None,
                                    stage_pool=ocp))
            for nn in range(4):
                _drain(_outproj_gen(nc, cx, bl, nn, 0, half=1, pool=apsum))

    nc.compile()
    return nc


def _host_prep(x_norm, Wqkv, bqkv, Wout, bout, b, t, c, h, d, n_cores):
    """Build per-core input maps (numpy, bf16)."""
    HL = h // n_cores
    R = b * t
    perm = np.concatenate([np.arange(0, d, 2), np.arange(1, d, 2)])  # deinterleave

    RC = 512
    XT = x_norm.reshape(R, c).T.astype(NPBF16)          # [C, R]
    # pre-tile: [R//RC, 128, KT*RC] so each row-chunk DMA is contiguous
    KT = c // 128
    xB = np.ascontiguousarray(
        XT.reshape(KT, 128, R // RC, RC).transpose(2, 1, 0, 3)
    ).reshape((R // RC) * 128, KT * RC)
    inv_freq = 1.0 / (ROPE_BASE ** (np.arange(0, d, 2, dtype=np.float64) / d))
    ang = np.arange(t, dtype=np.float64)[None, :] * inv_freq[:, None]  # [d/2, t]
    cosT = np.concatenate([np.cos(ang), np.cos(ang)], axis=0).astype(NPBF16)
    # sign-folded sin, laid out to match the *input* partition range of each
    # RoPE mul: rows 0-63 = +sin (multiplies x0 into rt[64:128]),
    # rows 64-127 = -sin (multiplies x1 into rt[0:64])
    sinT = np.concatenate([np.sin(ang), -np.sin(ang)], axis=0).astype(NPBF16)
    # upper-triangular (incl diagonal) 0/1 mask for the transposed P layout
    maskc = np.triu(np.ones((128, 128), dtype=np.float32)).astype(NPBF16)
    wo_bf = Wout.astype(NPBF16)                          # [C, C]
    wo_b = np.ascontiguousarray(
        wo_bf.reshape(KT, 128, c // 512, 512).transpose(2, 1, 0, 3)
    ).reshape((c // 512) * 128, KT * 512)

    in_maps = []
    for i in range(n_cores):
        cols_q = np.concatenate([i * HL * d + hh * d + perm for hh in range(HL)])
        cols_k = cols_q + h * d
        cols_v = np.concatenate([2 * h * d + i * HL * d + hh * d + np.arange(d)
                                 for hh in range(HL)])
        # pretile [C, HD] -> [128, KT*HD] (row p = concat over kt of W[kt*128+p, :])
        def _wtile(w):
            return np.ascontiguousarray(
                w.reshape(KT, 128, HL * d).transpose(1, 0, 2).reshape(128, KT * HL * d))
        wq_i = _wtile(Wqkv[:, cols_q].astype(NPBF16))
        wk_i = _wtile(Wqkv[:, cols_k].astype(NPBF16))
        wv_i = _wtile(Wqkv[:, cols_v].astype(NPBF16))
        bq_i = np.stack([bqkv[i * HL * d + hh * d + perm] for hh in range(HL)],
                        axis=1).astype(np.float32)
        bk_i = np.stack([bqkv[h * d + i * HL * d + hh * d + perm] for hh in range(HL)],
                        axis=1).astype(np.float32)
        # V bias folded into the attention epilogue (per-partition over d)
        bv_nat = bqkv[2 * h * d + i * HL * d:2 * h * d + (i + 1) * HL * d]
        bvh_i = np.stack([bv_nat[hh * d:(hh + 1) * d] for hh in range(HL)],
                         axis=1).astype(np.float32)
        in_maps.append({
            "xB": xB, "wq": wq_i, "wk": wk_i, "wv": wv_i,
            "bq": np.ascontiguousarray(bq_i), "bk": np.ascontiguousarray(bk_i),
            "bvh": np.ascontiguousarray(bvh_i),
            "wo": wo_b, "cosT": cosT, "sinT": sinT, "maskc": maskc,
        })
    return in_maps


def _gather(parts, b, t, c, n_cores):
    """Core j's out rows are, for each batch bb, global rows
    [bb*t + j*seg, bb*t + (j+1)*seg) with seg = t // n_cores."""
    seg = t // n_cores
    R = b * t
    full = np.empty((R, c), dtype=np.float32)
    for j in range(n_cores):
        for bb in range(b):
            full[bb * t + j * seg: bb * t + (j + 1) * seg] = \
                parts[j][bb * seg:(bb + 1) * seg]
    return full.reshape(b, t, c)


_NC_CACHE = {}


def kernel(x_norm, Wqkv, bqkv, Wout, bout):
    b, t, c = x_norm.shape
    h = 16
    d = c // h
    key = (b, t, c)
    if key not in _NC_CACHE:
        _NC_CACHE[key] = build_nc(b, t, c, h, d, N_CORES)
    nc = _NC_CACHE[key]
    in_maps = _host_prep(np.asarray(x_norm, dtype=np.float32),
                         np.asarray(Wqkv, dtype=np.float32),
                         np.asarray(bqkv, dtype=np.float32),
                         np.asarray(Wout, dtype=np.float32),
                         np.asarray(bout, dtype=np.float32),
                         b, t, c, h, d, N_CORES)
    res = run_bass_kernel_spmd(nc, in_maps, core_ids=list(range(N_CORES)))
    parts = [np.asarray(res.results[i]["out"], dtype=np.float32) for i in range(N_CORES)]
    full = _gather(parts, b, t, c, N_CORES)
    # bout is folded in host-side (the device-side drain is then a plain
    # scalar-engine copy, off the vector queue)
    full += np.asarray(bout, dtype=np.float32)[None, None, :]
    return full
